# revision 32
# baseline (speedup 1.0000x reference)
"""2-layer GAT (GATConv x2 + LayerNorm + ReLU) on Trainium2, 8-core SPMD.

Strategy (graph/data parallel):
  - Node halves are fixed by original id (v < N/2 -> half A -> cores 0-3).
    Within each half, dst nodes are sorted lexicographically by
    (cB, cA) = per-half in-degree counts and dealt round-robin to the
    half's 4 cores, so every 128-dst tile is nearly degree-homogeneous
    and the per-tile gather padding K = max in-tile degree stays close
    to the mean (~22 vs ~17 ideal, vs ~46 unsorted).
  - Per layer a packed bf16 node table is built sharded (each core
    computes rows for its own nodes: x @ W plus the attention
    projections) and assembled with an AllGather into pair-shared HBM:
      layer 1: [h+b1 (128) | a_s1 (4) | pad] 512B rows
      layer 2: [h2+b2 (64) | a_s2 (1) | pad] 256B rows
    a_d[dst] for a core's own nodes never leaves SBUF (stash).
  - dst tiles are packed into groups (sum K2 <= CAP cols) and each
    group's neighbor rows are fetched with two dma_gather calls (half A
    and half B tables; int16 indices) spread round-robin over 4 SWDGE
    queues so Q7 descriptor generation overlaps 4-wide.
  - Softmax skips the max-subtraction (logits are O(5)); padded slots
    point at a row with a_s = -1e9 so exp() kills them exactly.
  - LayerNorm uses fused DVE ops (tensor_tensor_reduce /
    scalar_tensor_tensor) and a per-group batched sqrt on the Scalar
    engine to avoid activation-table thrash against exp().
"""

import os
import types
from contextlib import ExitStack

import ml_dtypes
import numpy as np

import concourse.bass as bass
import concourse.mybir as mybir
import concourse.tile as tile
from concourse import bacc
from concourse.bass import AP
from concourse.masks import make_identity

F32 = mybir.dt.float32
BF16 = mybir.dt.bfloat16
I16 = mybir.dt.int16
AX = mybir.AxisListType
OP = mybir.AluOpType
ACT = mybir.ActivationFunctionType

# ---------------------------------------------------------------- problem cfg
N = 50000
E = 800000
IN_DIM = 128
HID = 32
HEADS = 4
EMB = 64
NEG = 0.2
EPS = 1e-5
NCORE = 8
TILES = 49
NPC = TILES * 128            # 6272 slots per core
NPAD = NCORE * NPC
SPLIT = NPAD // 2
REAL_PC = N // NCORE         # 6250 real nodes per core
PADROW = REAL_PC             # -1e9 row (relative to half base)
TB1 = 256                    # bf16 cols: h+b1(128) as(4) pad -> 512B rows
TB2 = 128                    # bf16 cols: h2+b2(64) as2(1) pad -> 256B rows
FB1 = IN_DIM + 2 * HEADS     # 136 matmul cols (h | as | ad)
FB2 = EMB + 2                # 66
NEGBIG = -1e9
CAP = 44                     # max sum(KA+KB) columns per gather group
HCAP = 26                    # max per-half columns per gather (ring depth)


# ------------------------------------------------------------------ host prep
def host_prep(edge_index):
    src = np.concatenate([np.asarray(edge_index[0]),
                          np.arange(N, dtype=np.int64)])
    dst = np.concatenate([np.asarray(edge_index[1]),
                          np.arange(N, dtype=np.int64)])
    srcA = src < N // 2
    cA = np.bincount(dst[srcA], minlength=N)
    cB = np.bincount(dst[~srcA], minlength=N)

    # per-half lexicographic (cB major, cA minor) descending ordering
    newid = np.empty(N, np.int64)
    for half, lo in ((0, 0), (1, N // 2)):
        nodes = np.arange(lo, lo + N // 2)
        order = nodes[np.lexsort((-cA[nodes], -cB[nodes]))]
        r = np.arange(len(order))
        newid[order] = (half * 4 + r % 4) * NPC + r // 4
    new2old = np.full(NPAD, -1, np.int64)
    new2old[newid] = np.arange(N)

    # per-tile K (shared across cores: program is SPMD)
    tile_of = (newid % NPC) // 128
    KA = np.zeros(TILES, np.int64)
    KB = np.zeros(TILES, np.int64)
    np.maximum.at(KA, tile_of, cA)
    np.maximum.at(KB, tile_of, cB)
    assert KA.min() > 0 and KB.min() > 0

    # greedy tile grouping: bound per-half gather columns (SWDGE ring
    # holds 1024 descs/DMA-engine; keep each gather well under) and the
    # SBUF column cap
    groups = []
    cur = []
    acca = accb = 0
    for t in range(TILES):
        ka, kb = int(KA[t]), int(KB[t])
        if cur and (acca + ka > HCAP or accb + kb > HCAP
                    or acca + accb + ka + kb > CAP):
            groups.append(cur)
            cur = []
            acca = accb = 0
        cur.append(t)
        acca += ka
        accb += kb
    groups.append(cur)
    GKA = [int(sum(KA[t] for t in g)) for g in groups]
    GKB = [int(sum(KB[t] for t in g)) for g in groups]

    # index stream: per group [A-chunks of tiles | B-chunks of tiles]
    # column offsets (in slot-columns of 128 idxs)
    aoff = {}
    boff = {}
    colA = []
    colB = []
    pos = 0
    for gi, g in enumerate(groups):
        colA.append(pos)
        for t in g:
            aoff[t] = pos
            pos += int(KA[t])
        colB.append(pos)
        for t in g:
            boff[t] = pos
            pos += int(KB[t])
    total_cols = pos
    total16 = total_cols * 128

    sn = newid[src]
    dn = newid[dst]
    core = dn // NPC
    slot = dn % NPC
    tl = slot // 128
    part = slot % 128
    # run position of each edge within its (dst, srchalf) bucket
    key = dn * 2 + (~srcA)
    eo = np.argsort(key, kind="stable")
    ks = key[eo]
    starts = np.r_[0, np.flatnonzero(np.diff(ks)) + 1]
    runlen = np.diff(np.r_[starts, len(ks)])
    runpos = np.arange(len(ks)) - np.repeat(starts, runlen)
    sne = sn[eo]
    ce = core[eo]
    te = tl[eo]
    pe = part[eo]
    loe = srcA[eo]

    idx16 = np.full((NCORE, total16), PADROW, np.int16)
    acol = np.vectorize(aoff.get)(te)
    bcol = np.vectorize(boff.get)(te)
    colpos = np.where(loe, (acol + runpos) * 128 + pe,
                      (bcol + runpos) * 128 + pe)
    val = np.where(loe, sne, sne - SPLIT).astype(np.int16)
    idx16[ce, colpos] = val

    w = idx16.reshape(NCORE, total16 // 16, 16).transpose(0, 2, 1)
    idx16_w = np.tile(w, (1, 8, 1))

    return types.SimpleNamespace(
        new2old=new2old, newid=newid,
        KA=KA.astype(int), KB=KB.astype(int),
        groups=groups, GKA=GKA, GKB=GKB,
        aoff=aoff, boff=boff, colA=colA, colB=colB,
        c16=total16 // 16, idx16=idx16_w,
    )


def host_weights(inputs):
    W1 = np.asarray(inputs["W1"], np.float32)
    W2 = np.asarray(inputs["W2"], np.float32)
    as1 = np.asarray(inputs["att_src1"], np.float32)
    ad1 = np.asarray(inputs["att_dst1"], np.float32)
    as2 = np.asarray(inputs["att_src2"], np.float32)
    ad2 = np.asarray(inputs["att_dst2"], np.float32)
    W1r = W1.reshape(IN_DIM, HEADS, HID)
    w_as1 = np.einsum("fhc,hc->fh", W1r, as1)
    w_ad1 = np.einsum("fhc,hc->fh", W1r, ad1)
    W1ext = np.concatenate([W1, w_as1, w_ad1], axis=1)            # [128,136]
    W2ext = np.concatenate([W2, W2 @ as2[0][:, None], W2 @ ad2[0][:, None]],
                           axis=1)                                # [128,66]
    b1e = np.zeros((1, FB1), np.float32)
    b1e[0, :IN_DIM] = np.asarray(inputs["b1"], np.float32)
    b2e = np.zeros((1, FB2), np.float32)
    b2e[0, :EMB] = np.asarray(inputs["b2"], np.float32)
    return {
        "W1ext": np.ascontiguousarray(W1ext.astype(np.float32)),
        "W2ext": np.ascontiguousarray(W2ext.astype(np.float32)),
        "B1E": np.tile(b1e, (128, 1)),
        "B2E": np.tile(b2e, (128, 1)),
        "G1": np.tile(np.asarray(inputs["gamma1"], np.float32), (128, 1)),
        "Be1": np.tile(np.asarray(inputs["beta1"], np.float32), (128, 1)),
        "G2": np.tile(np.asarray(inputs["gamma2"], np.float32), (128, 1)),
        "Be2": np.tile(np.asarray(inputs["beta2"], np.float32), (128, 1)),
    }


def host_xt(prep, x):
    """Per-core [IN_DIM, NPC] bf16 shard of x^T in slot order."""
    xt = np.zeros((IN_DIM, NPAD), np.float32)
    xt[:, prep.newid] = np.asarray(x, np.float32).T
    xt = xt.reshape(IN_DIM, NCORE, NPC).transpose(1, 0, 2)
    return [np.ascontiguousarray(xt[c]).astype(ml_dtypes.bfloat16)
            for c in range(NCORE)]


# ----------------------------------------------------------------- AP helpers
def apv(ap: AP, dims):
    """Replace the free dims of `ap` with explicit [step, count] pairs."""
    return AP(ap.tensor, ap.offset, [list(ap.ap[0])] + [list(d) for d in dims])


# ------------------------------------------------------------- device program
def build_program(prep, ln1_id=False, ln2_id=False):
    maxphase = int(os.environ.get("GAT_MAXPHASE", "4"))
    nc = bacc.Bacc("TRN2", target_bir_lowering=False, debug=False,
                   num_devices=NCORE, num_swdge_queues=4)
    KA, KB = prep.KA, prep.KB

    XT = nc.dram_tensor("xt", [IN_DIM, NPC], BF16, kind="ExternalInput")
    W1e = nc.dram_tensor("w1ext", [IN_DIM, FB1], F32, kind="ExternalInput")
    W2e = nc.dram_tensor("w2ext", [IN_DIM, FB2], F32, kind="ExternalInput")
    IDX16 = nc.dram_tensor("idx16", [128, prep.c16], I16, kind="ExternalInput")
    CB = {}
    for nm, cols in [("B1E", FB1), ("G1", IN_DIM), ("Be1", IN_DIM),
                     ("B2E", FB2), ("G2", EMB), ("Be2", EMB)]:
        CB[nm] = nc.dram_tensor(nm.lower(), [128, cols], F32,
                                kind="ExternalInput")
    OUT = nc.dram_tensor("out", [NPC, EMB], F32, kind="ExternalOutput")
    debug = bool(int(os.environ.get("GAT_DEBUG", "0")))
    if debug:
        D1 = nc.dram_tensor("d_t1b", [NPAD, TB1], BF16, kind="ExternalOutput")
        D2 = nc.dram_tensor("d_t2sh", [NPC, TB2], BF16, kind="ExternalOutput")
        D3 = nc.dram_tensor("d_ad1", [128, TILES * HEADS], F32,
                            kind="ExternalOutput")

    qn = [0]

    def next_q():
        q = qn[0]
        qn[0] = (q + 1) % 4
        return q

    with tile.TileContext(nc, num_cores=NCORE) as tc, ExitStack() as ctx:
        dram = ctx.enter_context(tc.tile_pool(name="dram", bufs=1,
                                              space="DRAM"))
        t1sh = dram.tile([NPC, TB1], BF16, name="t1sh")
        t2sh = dram.tile([NPC, TB2], BF16, name="t2sh")
        t1b = dram.tile([NPAD, TB1], BF16, name="t1b", addr_space="Shared")
        t2b = dram.tile([NPAD, TB2], BF16, name="t2b", addr_space="Shared")
        cpool = ctx.enter_context(tc.tile_pool(name="const", bufs=1))
        w1s = cpool.tile([IN_DIM, FB1], BF16, name="w1s")
        w2s = cpool.tile([IN_DIM, FB2], BF16, name="w2s")
        w1f = cpool.tile([IN_DIM, FB1], F32, name="w1f")
        w2f = cpool.tile([IN_DIM, FB2], F32, name="w2f")
        nc.sync.dma_start(w1f[:], W1e[:])
        nc.sync.dma_start(w2f[:], W2e[:])
        nc.vector.tensor_copy(w1s[:], w1f[:])
        nc.vector.tensor_copy(w2s[:], w2f[:])
        cb = {}
        for nm in CB:
            cb[nm] = cpool.tile(list(CB[nm].shape), F32, name=f"sb_{nm}")
            nc.sync.dma_start(cb[nm][:], CB[nm][:])
        ident = cpool.tile([128, 128], F32, name="ident")
        make_identity(nc, ident[:])
        negb = cpool.tile([32, 8], BF16, name="negb")
        nc.vector.memset(negb[:], NEGBIG)
        padr2 = cpool.tile([32, EMB + 1], BF16, name="padr2")
        nc.vector.memset(padr2[:], 0.0)
        nc.vector.memset(padr2[:, EMB:EMB + 1], NEGBIG)
        epst = cpool.tile([128, 1], F32, name="epst")
        nc.vector.memset(epst[:], EPS)
        i16b = cpool.tile([128, prep.c16], I16, name="i16b")
        nc.sync.dma_start(i16b[:], IDX16[:])
        ad1o = cpool.tile([128, TILES, HEADS], F32, name="ad1o")
        ad2o = cpool.tile([128, TILES], F32, name="ad2o")
        if maxphase < 4:
            dummy = cpool.tile([128, EMB], F32, name="dummy")
            nc.vector.memset(dummy[:], 0.5)
            nc.sync.dma_start(OUT[0:128, :], dummy[:])

        # ---------------- phase 1: layer-1 table shard + a_d stash
        with tc.tile_pool(name="ph1", bufs=2) as ph1, \
             tc.tile_pool(name="ph1p", bufs=4, space="PSUM") as ph1p:
            xts = ph1.tile([IN_DIM, NPC], BF16, tag="xts")
            nc.sync.dma_start(xts[:], XT[:])
            for t in range(TILES):
                ps = ph1p.tile([128, FB1], F32, tag="ps")
                nc.tensor.matmul(ps[:], lhsT=xts[:, t * 128:(t + 1) * 128],
                                 rhs=w1s[:], start=True, stop=True)
                stage = ph1.tile([128, IN_DIM + HEADS], BF16, tag="stage")
                nc.vector.tensor_tensor(stage[:], ps[:, 0:IN_DIM + HEADS],
                                        cb["B1E"][:, 0:IN_DIM + HEADS], OP.add)
                nc.scalar.activation(ad1o[:, t, :], ps[:, IN_DIM + HEADS:FB1],
                                     ACT.Copy)
                nc.sync.dma_start(
                    t1sh[t * 128:(t + 1) * 128, 0:IN_DIM + HEADS], stage[:])
        nc.sync.dma_start(
            t1sh[REAL_PC:REAL_PC + 22, IN_DIM:IN_DIM + HEADS],
            negb[0:22, 0:HEADS])
        nc.gpsimd.collective_compute(
            "AllGather", OP.bypass, replica_groups=[list(range(NCORE))],
            ins=[t1sh[:].opt()], outs=[t1b[:].opt()])
        if debug:
            nc.sync.dma_start(D1[:], t1b[:])
            nc.sync.dma_start(D3[:], ad1o[:].opt())

        # ---------------- phase 2: layer 1 per group; build layer-2 shard
        k2max = int(max(KA[t] + KB[t] for t in range(TILES)))
        with tc.tile_pool(name="gp", bufs=7) as gp, \
             tc.tile_pool(name="spp", bufs=3) as spp, \
             tc.tile_pool(name="sps", bufs=4) as sps, \
             tc.tile_pool(name="pp", bufs=4, space="PSUM") as pp:
            for gi, g in (list(enumerate(prep.groups)) if maxphase >= 2 else []):
                gka, gkb = prep.GKA[gi], prep.GKB[gi]
                gk2 = gka + gkb
                G = gp.tile([128, gk2, TB1], BF16, tag="G")
                nc.gpsimd.dma_gather(
                    G[:, 0:gka, :], t1b[0:SPLIT, :],
                    i16b[:, prep.colA[gi] * 8:(prep.colA[gi] + gka) * 8],
                    gka * 128, gka * 128, TB1, single_packet=False,
                    queue_num=next_q())
                nc.gpsimd.dma_gather(
                    G[:, gka:gk2, :], t1b[SPLIT:NPAD, :],
                    i16b[:, prep.colB[gi] * 8:(prep.colB[gi] + gkb) * 8],
                    gkb * 128, gkb * 128, TB1, single_packet=False,
                    queue_num=next_q())
                base = prep.colA[gi]

                ng = len(g)
                h1f = [None] * ng
                mus = spp.tile([128, ng], F32, tag="mus")
                nvs = spp.tile([128, ng], F32, tag="nvs")
                for ti, t in enumerate(g):
                    ka, kb, k2 = int(KA[t]), int(KB[t]), int(KA[t] + KB[t])
                    ao = prep.aoff[t] - base
                    bo = prep.boff[t] - base
                    ef = sps.tile([128, k2max, HEADS], F32, tag="e")
                    e = ef[:, 0:k2, :]
                    asA = apv(G[:, ao, IN_DIM:IN_DIM + HEADS],
                              [[TB1, ka], [1, HEADS]])
                    asB = apv(G[:, bo, IN_DIM:IN_DIM + HEADS],
                              [[TB1, kb], [1, HEADS]])
                    adv = apv(ad1o[:, t, :], [[0, ka], [1, HEADS]])
                    nc.vector.tensor_tensor(e[:, 0:ka, :], asA, adv, OP.add)
                    adv2 = apv(ad1o[:, t, :], [[0, kb], [1, HEADS]])
                    nc.vector.tensor_tensor(e[:, ka:k2, :], asB, adv2, OP.add)
                    # leaky relu + exp (no max subtraction; logits are small)
                    nc.vector.scalar_tensor_tensor(
                        e[:], e[:], NEG, e[:], OP.mult, OP.max)
                    nc.scalar.activation(e[:], e[:], ACT.Exp)
                    den = sps.tile([128, HEADS], F32, tag="den")
                    e_hk = apv(e[:], [[1, HEADS], [HEADS, k2]])
                    nc.vector.reduce_sum(den[:], e_hk, axis=AX.X)
                    inv = sps.tile([128, HEADS], F32, tag="inv")
                    nc.vector.reciprocal(inv[:], den[:])
                    inv_b = apv(inv[:], [[0, k2], [1, HEADS]])
                    nc.vector.tensor_tensor(e[:], e[:], inv_b, OP.mult)
                    # weighted message sum over both chunks
                    ghA = apv(G[:, ao, 0:IN_DIM], [[TB1, ka], [1, IN_DIM]])
                    aA = apv(e[:, 0:ka, :], [[HEADS, ka], [1, HEADS], [0, HID]])
                    nc.vector.tensor_tensor(ghA, ghA, aA, OP.mult)
                    ghB = apv(G[:, bo, 0:IN_DIM], [[TB1, kb], [1, IN_DIM]])
                    aB = apv(e[:, ka:k2, :], [[HEADS, kb], [1, HEADS], [0, HID]])
                    nc.vector.tensor_tensor(ghB, ghB, aB, OP.mult)
                    h1a = sps.tile([128, IN_DIM], F32, tag="h1a")
                    h1b = sps.tile([128, IN_DIM], F32, tag="h1b")
                    nc.vector.reduce_sum(
                        h1a[:], apv(G[:, ao, 0:IN_DIM],
                                    [[1, IN_DIM], [TB1, ka]]), axis=AX.X)
                    nc.vector.reduce_sum(
                        h1b[:], apv(G[:, bo, 0:IN_DIM],
                                    [[1, IN_DIM], [TB1, kb]]), axis=AX.X)
                    hf = spp.tile([128, IN_DIM], F32, tag=f"hf{ti}")
                    nc.vector.scalar_tensor_tensor(
                        hf[:], h1a[:], 0.0, h1b[:], OP.add, OP.add,
                        accum_out=mus[:, ti:ti + 1])
                    h1f[ti] = hf
                nc.vector.tensor_scalar_mul(mus[:], mus[:], 1.0 / IN_DIM)
                for ti, t in enumerate(g):
                    sq = sps.tile([128, IN_DIM], F32, tag="sq")
                    nc.vector.scalar_tensor_tensor(
                        sq[:], h1f[ti][:], mus[:, ti:ti + 1], h1f[ti][:],
                        OP.subtract, OP.mult, accum_out=nvs[:, ti:ti + 1])
                # batched LN tail for the group (one sqrt table load)
                std = sps.tile([128, ng], F32, tag="std")
                nc.scalar.activation(std[:], nvs[:], ACT.Sqrt,
                                     bias=epst[:], scale=1.0 / IN_DIM)
                rstd = spp.tile([128, ng], F32, tag="rstd")
                nc.vector.reciprocal(rstd[:], std[:])
                for ti, t in enumerate(g):
                    hf = h1f[ti]
                    y2 = sps.tile([128, IN_DIM], F32, tag="y2")
                    if ln1_id:
                        nc.vector.tensor_scalar(
                            y2[:], hf[:], mus[:, ti:ti + 1],
                            rstd[:, ti:ti + 1], OP.subtract, OP.mult)
                    else:
                        y = sps.tile([128, IN_DIM], F32, tag="y")
                        nc.vector.scalar_tensor_tensor(
                            y[:], hf[:], mus[:, ti:ti + 1], cb["G1"][:],
                            OP.subtract, OP.mult)
                        nc.vector.scalar_tensor_tensor(
                            y2[:], y[:], rstd[:, ti:ti + 1], cb["Be1"][:],
                            OP.mult, OP.add)
                    y3 = sps.tile([128, IN_DIM], F32, tag="y3")
                    nc.scalar.activation(y3[:], y2[:], ACT.Relu)
                    pst = pp.tile([128, 128], F32, tag="pst")
                    nc.tensor.transpose(pst[:], y3[:], ident[:])
                    h1t = sps.tile([128, 128], BF16, tag="h1t")
                    nc.scalar.activation(h1t[:], pst[:], ACT.Copy)
                    ps2 = pp.tile([128, FB2], F32, tag="ps2")
                    nc.tensor.matmul(ps2[:], lhsT=h1t[:], rhs=w2s[:],
                                     start=True, stop=True)
                    t2row = sps.tile([128, EMB + 1], BF16, tag="t2r")
                    nc.vector.tensor_tensor(t2row[:], ps2[:, 0:EMB + 1],
                                            cb["B2E"][:, 0:EMB + 1], OP.add)
                    nc.scalar.activation(ad2o[:, t:t + 1],
                                          ps2[:, EMB + 1:EMB + 2], ACT.Copy)
                    nc.sync.dma_start(
                        t2sh[t * 128:(t + 1) * 128, 0:EMB + 1], t2row[:])
        nc.sync.dma_start(t2sh[REAL_PC:REAL_PC + 22, 0:EMB + 1],
                          padr2[0:22, :])
        if debug:
            nc.sync.dma_start(D2[:], t2sh[:])
        if maxphase >= 3:
            nc.gpsimd.collective_compute(
                "AllGather", OP.bypass, replica_groups=[list(range(NCORE))],
                ins=[t2sh[:].opt()], outs=[t2b[:].opt()])

        # ---------------- phase 4: layer 2
        with tc.tile_pool(name="gp2", bufs=10) as gp2, \
             tc.tile_pool(name="sp2p", bufs=3) as sp2p, \
             tc.tile_pool(name="sp2", bufs=4) as sp2:
            for gi, g in (list(enumerate(prep.groups)) if maxphase >= 4 else []):
                gka, gkb = prep.GKA[gi], prep.GKB[gi]
                gk2 = gka + gkb
                G = gp2.tile([128, gk2, TB2], BF16, tag="G2")
                nc.gpsimd.dma_gather(
                    G[:, 0:gka, :], t2b[0:SPLIT, :],
                    i16b[:, prep.colA[gi] * 8:(prep.colA[gi] + gka) * 8],
                    gka * 128, gka * 128, TB2, single_packet=False,
                    queue_num=next_q())
                nc.gpsimd.dma_gather(
                    G[:, gka:gk2, :], t2b[SPLIT:NPAD, :],
                    i16b[:, prep.colB[gi] * 8:(prep.colB[gi] + gkb) * 8],
                    gkb * 128, gkb * 128, TB2, single_packet=False,
                    queue_num=next_q())
                base = prep.colA[gi]

                ng = len(g)
                h2f = [None] * ng
                mus = sp2p.tile([128, ng], F32, tag="mus")
                nvs = sp2p.tile([128, ng], F32, tag="nvs")
                for ti, t in enumerate(g):
                    ka, kb, k2 = int(KA[t]), int(KB[t]), int(KA[t] + KB[t])
                    ao = prep.aoff[t] - base
                    bo = prep.boff[t] - base
                    ef = sp2.tile([128, k2max], F32, tag="e")
                    e = ef[:, 0:k2]
                    asA = apv(G[:, ao, EMB:EMB + 1], [[TB2, ka]])
                    asB = apv(G[:, bo, EMB:EMB + 1], [[TB2, kb]])
                    nc.vector.tensor_scalar_add(e[:, 0:ka], asA,
                                                ad2o[:, t:t + 1])
                    nc.vector.tensor_scalar_add(e[:, ka:k2], asB,
                                                ad2o[:, t:t + 1])
                    nc.vector.scalar_tensor_tensor(
                        e[:], e[:], NEG, e[:], OP.mult, OP.max)
                    nc.scalar.activation(e[:], e[:], ACT.Exp)
                    den = sp2.tile([128, 1], F32, tag="den")
                    nc.vector.reduce_sum(den[:], e[:], axis=AX.X)
                    inv = sp2.tile([128, 1], F32, tag="inv")
                    nc.vector.reciprocal(inv[:], den[:])
                    nc.vector.tensor_scalar_mul(e[:], e[:], inv[:])
                    ghA = apv(G[:, ao, 0:EMB], [[TB2, ka], [1, EMB]])
                    aA = apv(e[:, 0:ka], [[1, ka], [0, EMB]])
                    nc.vector.tensor_tensor(ghA, ghA, aA, OP.mult)
                    ghB = apv(G[:, bo, 0:EMB], [[TB2, kb], [1, EMB]])
                    aB = apv(e[:, ka:k2], [[1, kb], [0, EMB]])
                    nc.vector.tensor_tensor(ghB, ghB, aB, OP.mult)
                    h2a = sp2.tile([128, EMB], F32, tag="h2a")
                    h2b = sp2.tile([128, EMB], F32, tag="h2b")
                    nc.vector.reduce_sum(
                        h2a[:], apv(G[:, ao, 0:EMB],
                                    [[1, EMB], [TB2, ka]]), axis=AX.X)
                    nc.vector.reduce_sum(
                        h2b[:], apv(G[:, bo, 0:EMB],
                                    [[1, EMB], [TB2, kb]]), axis=AX.X)
                    hf = sp2p.tile([128, EMB], F32, tag=f"hf{ti}")
                    nc.vector.scalar_tensor_tensor(
                        hf[:], h2a[:], 0.0, h2b[:], OP.add, OP.add,
                        accum_out=mus[:, ti:ti + 1])
                    h2f[ti] = hf
                nc.vector.tensor_scalar_mul(mus[:], mus[:], 1.0 / EMB)
                for ti, t in enumerate(g):
                    sq = sp2.tile([128, EMB], F32, tag="sq")
                    nc.vector.scalar_tensor_tensor(
                        sq[:], h2f[ti][:], mus[:, ti:ti + 1], h2f[ti][:],
                        OP.subtract, OP.mult, accum_out=nvs[:, ti:ti + 1])
                std = sp2.tile([128, ng], F32, tag="std")
                nc.scalar.activation(std[:], nvs[:], ACT.Sqrt,
                                     bias=epst[:], scale=1.0 / EMB)
                rstd = sp2p.tile([128, ng], F32, tag="rstd")
                nc.vector.reciprocal(rstd[:], std[:])
                for ti, t in enumerate(g):
                    hf = h2f[ti]
                    y2 = sp2.tile([128, EMB], F32, tag="y2")
                    if ln2_id:
                        nc.vector.tensor_scalar(
                            y2[:], hf[:], mus[:, ti:ti + 1],
                            rstd[:, ti:ti + 1], OP.subtract, OP.mult)
                    else:
                        y = sp2.tile([128, EMB], F32, tag="y")
                        nc.vector.scalar_tensor_tensor(
                            y[:], hf[:], mus[:, ti:ti + 1], cb["G2"][:],
                            OP.subtract, OP.mult)
                        nc.vector.scalar_tensor_tensor(
                            y2[:], y[:], rstd[:, ti:ti + 1], cb["Be2"][:],
                            OP.mult, OP.add)
                    nc.sync.dma_start(OUT[t * 128:(t + 1) * 128, :], y2[:])

    nc.compile()
    return nc


# ------------------------------------------------------------------ execution
def make_in_maps(prep, inputs):
    wts = host_weights(inputs)
    xts = host_xt(prep, inputs["x"])
    in_maps = []
    for c in range(NCORE):
        m = {
            "xt": xts[c],
            "w1ext": wts["W1ext"], "w2ext": wts["W2ext"],
            "idx16": np.ascontiguousarray(prep.idx16[c]),
            "b1e": wts["B1E"], "b2e": wts["B2E"],
            "g1": wts["G1"], "be1": wts["Be1"],
            "g2": wts["G2"], "be2": wts["Be2"],
        }
        in_maps.append(m)
    return in_maps


def assemble(prep, outs):
    full = np.zeros((N, EMB), np.float32)
    for c in range(NCORE):
        o = outs[c]["out"]
        olds = prep.new2old[c * NPC:(c + 1) * NPC]
        valid = olds >= 0
        full[olds[valid]] = o[valid]
    return full


_CACHE = {}


def kernel(**inputs):
    from concourse.bass_utils import run_bass_kernel_spmd
    edge_index = np.asarray(inputs["edge_index"])
    if "prog" not in _CACHE:
        prep = host_prep(edge_index)
        ln1_id = (np.allclose(inputs["gamma1"], 1.0)
                  and np.allclose(inputs["beta1"], 0.0))
        ln2_id = (np.allclose(inputs["gamma2"], 1.0)
                  and np.allclose(inputs["beta2"], 0.0))
        nc = build_program(prep, ln1_id, ln2_id)
        _CACHE["prog"] = (prep, nc)
    prep, nc = _CACHE["prog"]
    in_maps = make_in_maps(prep, inputs)
    res = run_bass_kernel_spmd(
        nc, in_maps, core_ids=list(range(NCORE)),
        trace=bool(int(os.environ.get("GAT_TRACE", "0"))))
    out = assemble(prep, res.results)
    if res.exec_time_ns is not None:
        kernel.last_exec_time_ns = res.exec_time_ns
    return out


kernel.last_exec_time_ns = None
